# revision 1
# baseline (speedup 1.0000x reference)
"""Trainium2 Bass kernel for nn_BPBookLayer (retrieval_knn).

Computation (per full input):
  query = mean(x, axis=1)                         [B, D]
  scores = cos_sim(query, prototypes)             [B, P]
  top5 -> softmax -> agg = attn @ protos[top5]    [B, D]
  out = x + 0.1 * agg[:, None, :]

Sharding: data-parallel over batch B=32 across 8 cores (4 batches/core),
prototypes replicated.

Per-core implementation notes:
 - q is computed with x tiles as the matmul *stationary* operand
   (lhsT) against a ones vector, giving qT columns (D on partitions)
   while x streams in from HBM; accumulated per quarter-batch so the
   PE work overlaps the DMA loads (PSUM allows only one accumulation
   group per 2KB bank, so each column group gets its own bank from a
   ping-pong pool and is copied out to SBUF).
 - top-5 is selection-free: 5th-largest score via DVE max8, mask
   scores >= t5, masked softmax over the full row, then
   agg = (e*mask) @ prototypes as a matmul (scaled 0.1/denom).
 - [128, 8] column layouts <-> [1, 1024] row layouts are converted
   with per-chunk one-shot PE transpose matmuls, each into its own
   ping-pong PSUM bank (PSUM allows one live accumulation group per
   2KB bank, and start=True logically zeroes the whole bank).
 - prototypes live in SBUF both raw [P, D] (for agg) and
   normalized-transposed [D, P] (for scores; built on-device with a
   diag(1/||p||)-scaled transpose matmul).
 - final residual add on DVE over the SBUF-staged batch.
"""

from contextlib import ExitStack

import numpy as np

import concourse.bacc as bacc
import concourse.bass as bass
import concourse.tile as tile
from concourse import mybir
from concourse.bass_utils import run_bass_kernel_spmd
from concourse.masks import make_identity

F32 = mybir.dt.float32
F32R = mybir.dt.float32r
AF = mybir.ActivationFunctionType
ALU = mybir.AluOpType

B, L, D, P = 32, 2048, 1024, 1024
NCORES = 8
BLOC = B // NCORES  # batches per core
TROWS = 256  # L rows per x tile
TSUB = TROWS // 128
NT = L // TROWS     # x tiles per batch
DCH = D // 128      # d chunks
PCH = P // 128      # p chunks
NQ = 8   # groups for the q accumulation
XBUFS = 16
ALPHA = 0.1


def _kernel(tc, ctx, x, protos, out, repeat=1):
    nc = tc.nc

    singles = ctx.enter_context(tc.tile_pool(name="singles", bufs=1))
    xp = ctx.enter_context(tc.tile_pool(name="xp", bufs=XBUFS))
    sm = ctx.enter_context(tc.tile_pool(name="sm", bufs=2))
    # single-column / small accumulators, one PSUM bank each
    ps_col = ctx.enter_context(tc.tile_pool(name="ps_col", bufs=4, space="PSUM"))
    ps_bc = ctx.enter_context(tc.tile_pool(name="ps_bc", bufs=2, space="PSUM"))

    for _rep in range(repeat):
        # ---- constants ----
        ident = singles.tile([128, 128], F32)
        make_identity(nc, ident)
        ones_col = singles.tile([128, 1], F32)
        nc.vector.memset(ones_col, 1.0)
        ones_row = singles.tile([1, 128], F32)
        nc.vector.memset(ones_row, 1.0)
        ones128 = singles.tile([128, 128], F32)
        nc.vector.memset(ones128, 1.0)

        # ---- batch-0 x loads first: they gate the first chain, while the
        # prototype pipeline below them on the same ring overlaps it ----
        # ---- prototypes + batch-0 x, interleaved on the load ring; per-chunk
        # setup pipeline: chunk DMA -> sq-norm -> rsqrt -> diag -> 8 transposes
        proto_sb = singles.tile([128, PCH, D], F32)
        protoT_sb = singles.tile([128, DCH, P], F32)

        inv_pnorm = singles.tile([128, PCH], F32)
        pnorm_sq = singles.tile([128, PCH], F32)

        xt_first = []
        sq_scratch = sm.tile([128, D], F32, tag="agg", bufs=1)
        for c in range(PCH):
            t_ = xp.tile([128, TSUB, D], F32, tag="x", name=f"x0_{c}")
            xt_first.append(t_)
            nc.sync.dma_start(
                out=t_,
                in_=x[0, TROWS * c : TROWS * (c + 1), :].rearrange(
                    "(t p) d -> p t d", p=128
                ),
            )
            nc.sync.dma_start(
                out=proto_sb[:, c, :],
                in_=protos[c * 128 : (c + 1) * 128, :],
            )
            nc.scalar.activation(
                out=sq_scratch,
                in_=proto_sb[:, c, :],
                func=AF.Square,
                accum_out=pnorm_sq[:, c : c + 1],
            )
            nc.scalar.activation(
                out=inv_pnorm[:, c : c + 1], in_=pnorm_sq[:, c : c + 1], func=AF.Sqrt
            )
            nc.vector.reciprocal(
                out=inv_pnorm[:, c : c + 1], in_=inv_pnorm[:, c : c + 1]
            )
            # protoT_n[d, p] = proto[p, d] / ||proto_p||  via lhsT.T @ diag
            diag_c = sm.tile([128, 128], F32, tag="diag", bufs=2, name=f"diag_{c}")
            nc.vector.tensor_scalar_mul(diag_c, ident, inv_pnorm[:, c : c + 1])
            for dc in range(DCH):
                pst = ps_col.tile([128, 128], F32, tag="col")
                nc.tensor.matmul(
                    pst,
                    lhsT=proto_sb[:, c, dc * 128 : (dc + 1) * 128],
                    rhs=diag_c,
                    start=True,
                    stop=True,
                )
                if dc % 2 == 0:
                    nc.scalar.copy(
                        out=protoT_sb[:, dc, c * 128 : (c + 1) * 128], in_=pst
                    )
                else:
                    nc.vector.tensor_copy(
                        protoT_sb[:, dc, c * 128 : (c + 1) * 128], pst
                    )

        # ---- per batch ----
        TPQ = NT // NQ  # x tiles per q accumulation group
        for b in range(BLOC):
            xt = []
            qq_sb = sm.tile([128, NQ, DCH], F32, tag="qq")
            for quarter in range(NQ):
                for i in range(quarter * TPQ, (quarter + 1) * TPQ):
                    if b == 0:
                        xt.append(xt_first[i])
                    else:
                        t_ = xp.tile([128, TSUB, D], F32, tag="x")
                        xt.append(t_)
                        nc.sync.dma_start(
                            out=t_,
                            in_=x[b, TROWS * i : TROWS * (i + 1), :].rearrange(
                                "(t p) d -> p t d", p=128
                            ),
                        )
                # accumulate group qT columns: lhsT = x tile chunk vs ones
                for dc in range(DCH):
                    qcol = ps_col.tile([128, 1], F32, tag="col")
                    for ii in range(TPQ):
                        i = quarter * TPQ + ii
                        for t in range(TSUB):
                            nc.tensor.matmul(
                                qcol,
                                lhsT=xt[i][:, t, dc * 128 : (dc + 1) * 128],
                                rhs=ones_col,
                                start=(ii == 0 and t == 0),
                                stop=(ii == TPQ - 1 and t == TSUB - 1),
                            )
                    nc.scalar.copy(out=qq_sb[:, quarter, dc : dc + 1], in_=qcol)

            qT_sb = sm.tile([128, DCH], F32, tag="qT")
            nc.vector.tensor_add(qT_sb, qq_sb[:, 0, :], qq_sb[:, 1, :])
            for g in range(2, NQ):
                nc.vector.tensor_add(qT_sb, qT_sb, qq_sb[:, g, :])

            # ||q||: sum of squares over d (free-dim partial then partition matmul)
            qsq_sc = sm.tile([128, DCH], F32, tag="qsq_sc")
            qsq = sm.tile([128, 1], F32, tag="qsq")
            nc.scalar.activation(
                out=qsq_sc, in_=qT_sb, func=AF.Square, accum_out=qsq
            )
            qn_ps = ps_col.tile([128, 1], F32, tag="col")
            nc.tensor.matmul(qn_ps, lhsT=ones128, rhs=qsq, start=True, stop=True)
            inv_qn = sm.tile([128, 1], F32, tag="inv_qn")
            nc.scalar.activation(out=inv_qn, in_=qn_ps, func=AF.Sqrt)
            nc.vector.reciprocal(out=inv_qn, in_=inv_qn)

            # scoresT[p, 1] = protoT_n.T @ qT  (raw q; proto already normalized)
            st_sb = sm.tile([128, PCH], F32, tag="st")
            for c in range(PCH):
                scol = ps_col.tile([128, 1], F32, tag="col")
                for dc in range(DCH):
                    nc.tensor.matmul(
                        scol,
                        lhsT=protoT_sb[:, dc, c * 128 : (c + 1) * 128],
                        rhs=qT_sb[:, dc : dc + 1],
                        start=(dc == 0),
                        stop=(dc == DCH - 1),
                    )
                nc.vector.tensor_copy(st_sb[:, c : c + 1], scol)

            # columns -> one scores row [1, P] via per-chunk PE transpose
            scores_sb = sm.tile([1, P], F32, tag="scores", bufs=1)
            for c in range(PCH):
                tr_ps = ps_col.tile([1, 128], F32, tag="col")
                nc.tensor.matmul(
                    tr_ps, lhsT=st_sb[:, c : c + 1], rhs=ident, start=True, stop=True
                )
                if c % 2 == 0:
                    nc.scalar.copy(out=scores_sb[0:1, c * 128 : (c + 1) * 128], in_=tr_ps)
                else:
                    nc.vector.tensor_copy(scores_sb[0:1, c * 128 : (c + 1) * 128], tr_ps)

            # top-8 values (descending); t5 = 5th largest
            vals = sm.tile([1, 8], F32, tag="vals")
            nc.vector.max(out=vals, in_=scores_sb)

            # eT = exp(scoresT / ||q||) in column space (cos <= 1, no overflow;
            # softmax shift-free). Same fp values as a row-space exp would give.
            eT = sm.tile([128, PCH], F32, tag="eT")
            nc.scalar.activation(out=eT, in_=st_sb, func=AF.Exp, scale=inv_qn)

            # denominator from the top-5 values directly
            evals = sm.tile([1, 8], F32, tag="evals")
            nc.scalar.activation(
                out=evals, in_=vals, func=AF.Exp, scale=inv_qn[0:1, :]
            )
            den = sm.tile([1, 1], F32, tag="den")
            nc.vector.reduce_sum(out=den, in_=evals[0:1, 0:5], axis=mybir.AxisListType.X)
            coef = sm.tile([1, 1], F32, tag="coef")
            nc.vector.reciprocal(out=coef, in_=den)
            nc.scalar.mul(out=coef, in_=coef, mul=ALPHA)

            # broadcast t5 over partitions, mask and weight in column space
            t5_ps = ps_col.tile([128, 1], F32, tag="col")
            nc.tensor.matmul(
                t5_ps, lhsT=ones_row, rhs=vals[0:1, 4:5], start=True, stop=True
            )
            t5_col = sm.tile([128, 1], F32, tag="t5")
            nc.vector.tensor_copy(t5_col, t5_ps)
            wt_sb = sm.tile([128, PCH], F32, tag="wt")
            nc.vector.tensor_scalar(
                out=wt_sb,
                in0=st_sb,
                scalar1=t5_col,
                scalar2=None,
                op0=ALU.is_ge,
            )
            nc.vector.tensor_mul(wt_sb, wt_sb, eT)

            # aggT[d, 1] = proto.T @ wT  (raw prototypes, exact fp32)
            at_sb = sm.tile([128, DCH], F32, tag="at")
            for dc in range(DCH):
                acol = ps_col.tile([128, 1], F32, tag="col")
                for c in range(PCH):
                    nc.tensor.matmul(
                        acol,
                        lhsT=proto_sb[:, c, dc * 128 : (dc + 1) * 128],
                        rhs=wt_sb[:, c : c + 1],
                        start=(c == 0),
                        stop=(c == PCH - 1),
                    )
                nc.vector.tensor_copy(at_sb[:, dc : dc + 1], acol)

            # aggT columns -> agg row [1, D], scaled by 0.1/denom on the copies
            agg_sb = sm.tile([1, D], F32, tag="agg", bufs=1)
            for dc in range(DCH):
                ar_ps = ps_col.tile([1, 128], F32, tag="col")
                nc.tensor.matmul(
                    ar_ps, lhsT=at_sb[:, dc : dc + 1], rhs=ident, start=True, stop=True
                )
                nc.scalar.activation(
                    out=agg_sb[0:1, dc * 128 : (dc + 1) * 128],
                    in_=ar_ps,
                    func=AF.Copy,
                    scale=coef,
                )

            # broadcast (0.1/denom) * agg over 128 partitions
            bc_ps = ps_bc.tile([128, D], F32, tag="bc")
            for n in range(2):
                nc.tensor.matmul(
                    bc_ps[:, n * 512 : (n + 1) * 512],
                    lhsT=ones_row,
                    rhs=agg_sb[0:1, n * 512 : (n + 1) * 512],
                    start=True,
                    stop=True,
                )
            # out tiles = x tiles + bc (read straight from PSUM), then store
            bc_b = bc_ps.rearrange("p (o d) -> p o d", o=1).to_broadcast([128, TSUB, D])
            for i in range(NT):
                nc.vector.tensor_add(xt[i], xt[i], bc_b)
                nc.scalar.dma_start(
                    out=out[b, TROWS * i : TROWS * (i + 1), :].rearrange(
                        "(t p) d -> p t d", p=128
                    ),
                    in_=xt[i],
                )


def build_nc(repeat=1):
    nc = bacc.Bacc("TRN2", target_bir_lowering=False)
    x = nc.dram_tensor("x", [BLOC, L, D], F32, kind="ExternalInput")
    protos = nc.dram_tensor("prototypes", [P, D], F32, kind="ExternalInput")
    out = nc.dram_tensor("out", [BLOC, L, D], F32, kind="ExternalOutput")
    with tile.TileContext(nc) as tc, ExitStack() as ctx:
        _kernel(tc, ctx, x[:], protos[:], out[:], repeat=repeat)
    nc.finalize()
    return nc


def kernel(x, prototypes):
    x = np.ascontiguousarray(x, dtype=np.float32)
    prototypes = np.ascontiguousarray(prototypes, dtype=np.float32)
    assert x.shape == (B, L, D) and prototypes.shape == (P, D)
    nc = build_nc()
    in_maps = [
        {"x": x[c * BLOC : (c + 1) * BLOC], "prototypes": prototypes}
        for c in range(NCORES)
    ]
    res = run_bass_kernel_spmd(nc, in_maps, core_ids=list(range(NCORES)))
    return np.concatenate([r["out"] for r in res.results], axis=0)

